# revision 24
# baseline (speedup 1.0000x reference)
"""Trainium2 Bass kernel for nn_Decoder (3-layer GNN message-passing decoder).

Sharding: node axis split across 8 cores (2500 nodes/core), weights replicated.
Feature axis (C=128) lives on partitions; host pre-transposes to [C, ...] bf16.

v2 design (vs baseline): the LayerNorm rstd is computed on the Vector engine
with custom DVE ops (quadratic seed + 2 Newton steps, ~1.5e-4 rel err) instead
of Ln/Exp on the Scalar engine.  The ACT engine therefore only ever loads the
gelu table set (once), which lets the per-tile node phase (LN1/FFN/LN2) be
software-pipelined into the NEXT tile's edge phase with zero table thrash:
NODE(l,t) is emitted as ~10 small chunks spliced between the matmul/gelu
groups of EDGE(l,t+1).  LN applies and squares run on the otherwise-idle
GPSIMD engine.

Per-core, per-(layer,tile) structure (T=500-node tiles, K=32 edge slots):
  EDGE: S-broadcast add on DVE (PSUM->SBUF stg), gelu1 in 4000-elem ACT
        chunks, mm2 + gelu2 per 2-k PSUM group, mm3 PSUM-accumulated onto
        identity-preloaded h.
  NODE: x = acc + K*b3/30; stats via (1/C)-ones matmuls; var on DVE custom op;
        rstd via RSQRT_SEED + 2x RSQRT_NR (DVE); LN applies + mask on GPSIMD;
        FFN (di/do) matmuls on PE with gelu on ACT; next-layer S on PE.
"""

import sys
from contextlib import ExitStack

for _p in ("/opt/trn_rl_repo", "/root/.axon_site/_ro/trn_rl_repo"):
    if _p not in sys.path:
        sys.path.append(_p)

import numpy as np
import ml_dtypes

import concourse.bass as bass
import concourse.tile as tile
from concourse import bacc, mybir
from concourse.bass_utils import run_bass_kernel_spmd
from concourse.masks import make_identity
from concourse import dve_ops as _dvo
from concourse.dve_spec import (
    C0, C1, C2, Spec, Src0, Src1, lower as _dve_lower, _has_src1,
)
from concourse.dve_uop import DveOpSpec

N, K, C, H, L = 20000, 32, 128, 128, 3
NCORES = 8
NPER = N // NCORES          # 2500 nodes per core
T = 500                     # node tile
NT = NPER // T              # 5 tiles
SCALE, EPS = 30.0, 1e-5

BF = mybir.dt.bfloat16
F32 = mybir.dt.float32
AF = mybir.ActivationFunctionType
OP = mybir.AluOpType

# ---- custom DVE ops (registered once per process) ----

def _register_dve_op(name, spec, subdim=False):
    for o in _dvo.OPS:
        if o.name == name:
            return o
    row = max(_dvo._SUB_OPCODE_FOR_NAME.values()) + 1
    assert row < 0x20, "no free custom-DVE opcode rows"
    _dvo._SUB_OPCODE_FOR_NAME[name] = row
    shas = {}
    for ver in ("v3",):
        uops = _dve_lower(spec, ver=ver)
        shas[ver] = DveOpSpec(
            name=name, opcode=row, uops=uops, rd1_en=_has_src1(spec)
        ).sha(ver)
    op = _dvo.DveOp(name, spec, subdim=subdim, uops_sha=shas)
    _dvo.OPS.append(op)
    _dvo.CUSTOM_DVE_SPECS[name] = spec
    return op


# z0 = (c0*v + c1)*v + c2 : quadratic minimax seed for rsqrt on v in [0.35, 3]
RSQRT_SEED = _register_dve_op(
    "RSQRT_SEED_ANT",
    Spec(
        body=(C0 * Src0 + C1) * Src0 + C2,
        reference=lambda in0, in1, s0, s1, imm2: (
            (s0 * in0 + s1) * in0 + imm2
        ).astype(np.float32),
    ),
)
# z' = z*(1.5 - 0.5*v*z*z) : one rsqrt Newton step (Src0=v, Src1=z)
RSQRT_NR = _register_dve_op(
    "RSQRT_NR_ANT",
    Spec(
        body=Src1 * (C0 - C1 * (Src0 * (Src1 * Src1))),
        reference=lambda in0, in1, s0, s1, imm2: (
            in1 * (s0 - s1 * in0 * in1 * in1)
        ).astype(np.float32),
    ),
)
# u = in1 - in0*in0 + eps : variance from (mean, E[x^2]) with eps folded in
VAR_EPS = _register_dve_op(
    "VAR_EPS_ANT",
    Spec(
        body=(Src1 - Src0 * Src0) + C0,
        reference=lambda in0, in1, s0, s1, imm2: (
            in1 - in0 * in0 + s0
        ).astype(np.float32),
    ),
)

RS_C0, RS_C1, RS_C2 = 0.19236749, -0.99543426, 1.87889486


def _emit(ctx, tc, io, nper, tsz):
    nc = tc.nc
    nt = nper // tsz

    consts = ctx.enter_context(tc.tile_pool(name="consts", bufs=1))
    efpool = ctx.enter_context(tc.tile_pool(name="ef", bufs=2))
    stgpool = ctx.enter_context(tc.tile_pool(name="stg", bufs=2))
    m12pool = ctx.enter_context(tc.tile_pool(name="m12", bufs=3))
    spool = ctx.enter_context(tc.tile_pool(name="sp", bufs=4))
    meanpool = ctx.enter_context(tc.tile_pool(name="mean", bufs=3))
    upool = ctx.enter_context(tc.tile_pool(name="u", bufs=2))
    zpool = ctx.enter_context(tc.tile_pool(name="z", bufs=3))
    sqpool = ctx.enter_context(tc.tile_pool(name="sq", bufs=3))
    gpool = ctx.enter_context(tc.tile_pool(name="g", bufs=4))
    mdpool = ctx.enter_context(tc.tile_pool(name="md", bufs=2))
    outpool = ctx.enter_context(tc.tile_pool(name="out", bufs=2))
    papool = ctx.enter_context(tc.tile_pool(name="pa", bufs=2, space="PSUM"))
    pbpool = ctx.enter_context(tc.tile_pool(name="pb", bufs=2, space="PSUM"))
    psacc = ctx.enter_context(tc.tile_pool(name="psacc", bufs=1, space="PSUM"))
    psmisc = ctx.enter_context(tc.tile_pool(name="psmisc", bufs=1, space="PSUM"))

    # ---- persistent SBUF state ----
    ef00 = efpool.tile([C, K, tsz], BF, tag="ef", name="ef00")
    for q in range(16):
        nc.sync.dma_start(out=ef00[:, q * 2:(q + 1) * 2, :],
                          in_=io["efT"][:, q * 2:(q + 1) * 2, 0:tsz])
    nfh = consts.tile([C, nper], BF, tag="nfh")
    mask_rep = consts.tile([C, nper], BF, tag="maskr")
    h_sb = consts.tile([C, nper], BF, tag="hbuf")
    h1_sb = consts.tile([C, nper], BF, tag="h1")
    x2t = consts.tile([C, nper], BF, tag="x2t")

    # critical-path setup first: nfh + the weights needed for S(0,*) and the
    # first edge tile, so the first gelu lands as early as possible.  Large
    # host->SBUF transfers are split so they spread across DMA queues.
    for q in range(4):
        qs = slice(q * (nper // 4), (q + 1) * (nper // 4))
        nc.sync.dma_start(out=nfh[:, qs], in_=io["nfT"][:, qs])
    wts = {}
    for nm in ("w1aT", "w1bT", "w1eT", "w2T", "w3sT", "diwT", "dowT"):
        wt = consts.tile([C, L, H], BF, tag=nm, name=nm)
        for l in range(L):
            nc.sync.dma_start(out=wt[:, l, :], in_=io[nm][l, :, :])
        wts[nm] = wt
    bvec = consts.tile([C, 15], F32, tag="bvec")
    nc.sync.dma_start(out=bvec[:, :], in_=io["bvec"][:, :])
    lnvec = consts.tile([C, 12], F32, tag="lnvec")
    nc.sync.dma_start(out=lnvec[:, :], in_=io["lnvec"][:, :])

    ident = consts.tile([C, C], BF, tag="ident")
    make_identity(nc, ident[:, :])
    ones_sc = consts.tile([C, C], BF, tag="ones")
    nc.vector.memset(ones_sc[:, :], 1.0 / C)

    _m = io["maskT"]
    for q in range(4):
        q0 = q * (nper // 4)
        qs = slice(q0, q0 + nper // 4)
        _mb = bass.AP(tensor=_m.tensor, offset=_m.offset + q0,
                      ap=[[0, C], [_m.ap[1][0], nper // 4]])
        nc.sync.dma_start(out=mask_rep[:, qs], in_=_mb)

    def bcol(base, l):
        return bvec[:, base + l:base + l + 1]

    def lncol(base, l):
        return lnvec[:, base + l:base + l + 1]

    s_tiles = {}
    ef_tiles = {}
    acc_tiles = {}

    def emit_S(l, t, h_src):
        sl_ = slice(t * tsz, (t + 1) * tsz)
        s_ps = psmisc.tile([C, 512], F32, tag="pm", name="s_ps")
        nc.tensor.matmul(s_ps[:, 0:tsz], wts["w1aT"][:, l, :], h_src[:, sl_],
                         start=True, stop=False)
        nc.tensor.matmul(s_ps[:, 0:tsz], wts["w1bT"][:, l, :], nfh[:, sl_],
                         start=False, stop=True)
        s_sb = spool.tile([C, tsz], BF, tag="ssb", name="s_sb")
        nc.vector.tensor_copy(out=s_sb[:, :], in_=s_ps[:, 0:tsz])
        s_tiles[(l, t)] = s_sb

    def fetch_ef(l, t):
        if (l, t) in ef_tiles or t >= nt:
            return
        sl_ = slice(t * tsz, (t + 1) * tsz)
        ef_sb = efpool.tile([C, K, tsz], BF, tag="ef")
        for q in range(16):
            nc.sync.dma_start(out=ef_sb[:, q * 2:(q + 1) * 2, :],
                              in_=io["efT"][:, q * 2:(q + 1) * 2, sl_])
        ef_tiles[(l, t)] = ef_sb

    # ---- NODE(l, t): list of closures spliced into the next EDGE ----
    def node_chunks(l, t):
        sl = slice(t * tsz, (t + 1) * tsz)
        st = {}

        def ln_stats(src_bf):
            # -> (mean_sb f32, z f32) ; consumes one sq tile via gpsimd
            sq = sqpool.tile([C, tsz], BF, tag="sq", name="sq")
            nc.gpsimd.tensor_mul(sq[:, :], src_bf, src_bf)
            return sq

        def c0():
            acc_ps = acc_tiles.pop((l, t))
            nc.vector.tensor_scalar(x2t[:, sl], acc_ps[:, 0:tsz],
                                    bcol(6, l), None, OP.add)
            st["sq"] = ln_stats(x2t[:, sl])

        def c1():
            stp = psmisc.tile([C, 512], F32, tag="pm", name="st1")
            nc.tensor.matmul(stp[:, 0:tsz], ones_sc[:, :], x2t[:, sl],
                             start=True, stop=True)
            mean = meanpool.tile([C, tsz], F32, tag="mean", name="mean")
            nc.vector.tensor_copy(out=mean[:, :], in_=stp[:, 0:tsz])
            st["mean"] = mean

        def c2():
            stp = psmisc.tile([C, 512], F32, tag="pm", name="st2")
            nc.tensor.matmul(stp[:, 0:tsz], ones_sc[:, :], st["sq"][:, :],
                             start=True, stop=True)
            u = upool.tile([C, tsz], F32, tag="u", name="u")
            nc.vector._custom_dve(VAR_EPS, out=u[:, :], in0=st["mean"][:, :],
                                  in1=stp[:, 0:tsz], s0=EPS)
            z = zpool.tile([C, tsz], F32, tag="z", name="z")
            nc.vector._custom_dve(RSQRT_SEED, out=z[:, :], in0=u[:, :],
                                  s0=RS_C0, s1=RS_C1, imm2=RS_C2)
            nc.vector._custom_dve(RSQRT_NR, out=z[:, :], in0=u[:, :],
                                  in1=z[:, :], s0=1.5, s1=0.5)
            nc.vector._custom_dve(RSQRT_NR, out=z[:, :], in0=u[:, :],
                                  in1=z[:, :], s0=1.5, s1=0.5)
            st["z"] = z

        def c3():
            g1 = gpool.tile([C, tsz], BF, tag="g", name="g1")
            nc.gpsimd.tensor_sub(g1[:, :], x2t[:, sl], st["mean"][:, :])
            nc.gpsimd.tensor_mul(g1[:, :], g1[:, :], st["z"][:, :])
            nc.gpsimd.tensor_scalar(h1_sb[:, sl], g1[:, :],
                                    lncol(0, l), lncol(3, l), OP.mult, OP.add)

        def c4():
            dpa = psmisc.tile([C, 512], F32, tag="pm", name="dpa")
            nc.tensor.matmul(dpa[:, 0:tsz], wts["diwT"][:, l, :], h1_sb[:, sl],
                             start=True, stop=True)
            md = mdpool.tile([C, tsz], BF, tag="md", name="md")
            nc.scalar.activation(out=md[:, :], in_=dpa[:, 0:tsz], func=AF.Gelu,
                                 bias=bcol(9, l))
            st["md"] = md

        def c5():
            dpb = psmisc.tile([C, 512], F32, tag="pm", name="dpb")
            nc.tensor.matmul(dpb[:, 0:tsz], ident[:, :], h1_sb[:, sl],
                             start=True, stop=False)
            nc.tensor.matmul(dpb[:, 0:tsz], wts["dowT"][:, l, :],
                             st["md"][:, :], start=False, stop=True)
            nc.vector.tensor_scalar(x2t[:, sl], dpb[:, 0:tsz],
                                    bcol(12, l), None, OP.add)
            st["sq2"] = ln_stats(x2t[:, sl])

        def c6():
            stp = psmisc.tile([C, 512], F32, tag="pm", name="st1b")
            nc.tensor.matmul(stp[:, 0:tsz], ones_sc[:, :], x2t[:, sl],
                             start=True, stop=True)
            mean = meanpool.tile([C, tsz], F32, tag="mean", name="mean2")
            nc.vector.tensor_copy(out=mean[:, :], in_=stp[:, 0:tsz])
            st["mean2"] = mean

        def c7():
            stp = psmisc.tile([C, 512], F32, tag="pm", name="st2b")
            nc.tensor.matmul(stp[:, 0:tsz], ones_sc[:, :], st["sq2"][:, :],
                             start=True, stop=True)
            u = upool.tile([C, tsz], F32, tag="u", name="u2")
            nc.vector._custom_dve(VAR_EPS, out=u[:, :], in0=st["mean2"][:, :],
                                  in1=stp[:, 0:tsz], s0=EPS)
            z = zpool.tile([C, tsz], F32, tag="z", name="z2")
            nc.vector._custom_dve(RSQRT_SEED, out=z[:, :], in0=u[:, :],
                                  s0=RS_C0, s1=RS_C1, imm2=RS_C2)
            nc.vector._custom_dve(RSQRT_NR, out=z[:, :], in0=u[:, :],
                                  in1=z[:, :], s0=1.5, s1=0.5)
            nc.vector._custom_dve(RSQRT_NR, out=z[:, :], in0=u[:, :],
                                  in1=z[:, :], s0=1.5, s1=0.5)
            st["z2"] = z

        def c8():
            g2 = gpool.tile([C, tsz], BF, tag="g", name="g2")
            nc.gpsimd.tensor_sub(g2[:, :], x2t[:, sl], st["mean2"][:, :])
            nc.gpsimd.tensor_mul(g2[:, :], g2[:, :], st["z2"][:, :])
            if l < L - 1:
                q = gpool.tile([C, tsz], BF, tag="g", name="q")
                nc.gpsimd.tensor_scalar(q[:, :], g2[:, :],
                                        lncol(6, l), lncol(9, l),
                                        OP.mult, OP.add)
                nc.gpsimd.tensor_mul(h_sb[:, sl], q[:, :], mask_rep[:, sl])
            else:
                q = gpool.tile([C, tsz], BF, tag="g", name="q")
                nc.gpsimd.tensor_scalar(q[:, :], g2[:, :],
                                        lncol(6, l), lncol(9, l),
                                        OP.mult, OP.add)
                ot = outpool.tile([C, tsz], F32, tag="ot", name="ot")
                nc.gpsimd.tensor_mul(ot[:, :], q[:, :], mask_rep[:, sl])
                t0 = t * tsz
                for qq in range(4):
                    qs = slice(t0 + qq * (tsz // 4), t0 + (qq + 1) * (tsz // 4))
                    nc.sync.dma_start(out=io["out_hT"][:, qs],
                                      in_=ot[:, qq * (tsz // 4):(qq + 1) * (tsz // 4)])

        def c9():
            if l < L - 1:
                emit_S(l + 1, t, h_sb)

        return [c0, c1, c2, c3, c4, c5, c6, c7, c8, c9]

    pend = []

    def splice():
        if pend:
            pend.pop(0)()

    # ---- EDGE building blocks ----
    # A-half of tile (l,t): mm1e per k -> DVE S-add -> gelu1 per 8-k block.
    # Emitted interleaved: A1 inside B0, and the NEXT tile's A0 inside B1,
    # so gelu1 fills the ACT pipeline while pb slots recycle.
    a_state = {}

    def emit_A(l, t, h, sg):
        st = a_state[(l, t)]
        ef_sb, s_sb, m12 = st["ef"], st["s"], st["m12"]
        w1e = wts["w1eT"][:, l, :]
        stg = stgpool.tile([C, 8, tsz], F32, tag="stg", name="stg")
        for j in range(8):
            k0 = h * 16 + sg * 8 + j
            pa = papool.tile([C, 512], F32, tag="pa", name="pa")
            nc.tensor.matmul(pa[:, 0:tsz], w1e, ef_sb[:, k0, :],
                             start=True, stop=True)
            nc.vector.tensor_add(stg[:, j, 0:tsz], pa[:, 0:tsz], s_sb[:, :])
        nc.scalar.activation(out=m12[h][:, sg * 8:(sg + 1) * 8, 0:tsz],
                             in_=stg[:, :, 0:tsz],
                             func=AF.Gelu, bias=bcol(0, l))

    def open_tile(l, t):
        a_state[(l, t)] = dict(
            ef=ef_tiles.pop((l, t)),
            s=s_tiles.pop((l, t)),
            m12=[m12pool.tile([C, 16, tsz], BF, tag="m12", name=f"m12_{h}")
                 for h in range(2)],
        )

    def edge(l, t):
        sl = slice(t * tsz, (t + 1) * tsz)
        if t + 1 < nt:
            fetch_ef(l, t + 1)
        elif l + 1 < L:
            fetch_ef(l + 1, 0)
        h_cur = nfh if l == 0 else h_sb
        w2 = wts["w2T"][:, l, :]
        w3s = wts["w3sT"][:, l, :]
        m12 = a_state[(l, t)]["m12"]

        def phase_B(h, gr):
            for g in gr:
                pb = pbpool.tile([C, 2, 512], F32, tag="pb", name="pb")
                for i in range(2):
                    nc.tensor.matmul(pb[:, i, 0:tsz], w2,
                                     m12[h][:, g * 2 + i, 0:tsz],
                                     start=True, stop=True)
                nc.scalar.activation(out=m12[h][:, g * 2:(g + 1) * 2, 0:tsz],
                                     in_=pb[:, :, 0:tsz],
                                     func=AF.Gelu, bias=bcol(3, l))

        def phase_C(h, acc):
            for kk in range(16):
                nc.tensor.matmul(acc[:, 0:tsz], w3s, m12[h][:, kk, 0:tsz],
                                 start=False, stop=(h == 1 and kk == 15))

        # A0 of this tile was emitted during the previous tile's B1 (or in
        # the pipeline warmup for the very first tile).
        phase_B(0, range(0, 2))
        splice()
        phase_B(0, range(2, 4))
        emit_A(l, t, 1, 0)
        splice()
        phase_B(0, range(4, 6))
        emit_A(l, t, 1, 1)
        splice()
        phase_B(0, range(6, 8))
        splice()
        acc_ps = psacc.tile([C, 512], F32, tag="acc", name="acc_ps")
        nc.tensor.matmul(acc_ps[:, 0:tsz], ident[:, :], h_cur[:, sl],
                         start=True, stop=False)
        acc_tiles[(l, t)] = acc_ps
        phase_C(0, acc_ps)
        splice(); splice()
        # successor tile's A0 interleaves with our B1
        if t + 1 < nt:
            succ = (l, t + 1)
        elif l + 1 < L:
            succ = (l + 1, 0)
        else:
            succ = None
        phase_B(1, range(0, 2))
        splice()
        phase_B(1, range(2, 4))
        if succ is not None:
            open_tile(*succ)
            emit_A(*succ, 0, 0)
        splice()
        phase_B(1, range(4, 6))
        if succ is not None:
            emit_A(*succ, 0, 1)
        splice()
        phase_B(1, range(6, 8))
        splice()
        phase_C(1, acc_ps)

    # ---- main emission ----
    ef_tiles[(0, 0)] = ef00
    for t in range(nt):
        emit_S(0, t, nfh)
    open_tile(0, 0)
    emit_A(0, 0, 0, 0)
    emit_A(0, 0, 0, 1)
    for l in range(L):
        for t in range(nt):
            edge(l, t)
            pend.extend(node_chunks(l, t))
    while pend:
        pend.pop(0)()


def build_nc(nper=NPER, tsz=T):
    nc = bacc.Bacc("TRN2", target_bir_lowering=False, debug=False,
                   enable_asserts=False)
    io = {
        "efT": nc.dram_tensor("efT", [C, K, nper], BF, kind="ExternalInput").ap(),
        "nfT": nc.dram_tensor("nfT", [C, nper], BF, kind="ExternalInput").ap(),
        "maskT": nc.dram_tensor("maskT", [1, nper], BF, kind="ExternalInput").ap(),
        "bvec": nc.dram_tensor("bvec", [C, 15], F32, kind="ExternalInput").ap(),
        "lnvec": nc.dram_tensor("lnvec", [C, 12], F32, kind="ExternalInput").ap(),
        "out_hT": nc.dram_tensor("out_hT", [C, nper], F32, kind="ExternalOutput").ap(),
    }
    for nm in ("w1aT", "w1bT", "w1eT", "w2T", "w3sT", "diwT", "dowT"):
        io[nm] = nc.dram_tensor(nm, [L, C, H], BF, kind="ExternalInput").ap()
    with tile.TileContext(nc) as tc:
        with ExitStack() as ctx:
            _emit(ctx, tc, io, nper, tsz)
    nc.compile()
    return nc


def host_prep(inputs, nper=NPER, ncores=NCORES):
    """Shard + lay out inputs for the device. Returns list of per-core in_maps."""
    bf = ml_dtypes.bfloat16
    nf = np.asarray(inputs["node_features"], np.float32)
    ef = np.asarray(inputs["edge_features"], np.float32)
    mask = np.asarray(inputs["mask"], np.float32)
    w1 = np.asarray(inputs["w1"], np.float32)
    w2 = np.asarray(inputs["w2"], np.float32)
    w3 = np.asarray(inputs["w3"], np.float32)
    di_w = np.asarray(inputs["di_w"], np.float32)
    do_w = np.asarray(inputs["do_w"], np.float32)

    def tr(w):  # (L, A, B) -> (L, B, A) contiguous bf16
        return np.ascontiguousarray(w.transpose(0, 2, 1)).astype(bf)

    shared = {
        "w1aT": tr(w1[:, :, 0:C]),
        "w1bT": tr(w1[:, :, C:2 * C]),
        "w1eT": tr(w1[:, :, 3 * C:4 * C]),
        "w2T": tr(w2),
        "w3sT": tr(w3 / SCALE),
        "diwT": tr(di_w),
        "dowT": tr(do_w),
    }
    bvec = np.zeros((C, 15), np.float32)
    lnvec = np.zeros((C, 12), np.float32)
    for l in range(L):
        bvec[:, 0 + l] = np.asarray(inputs["b1"][l], np.float32)
        bvec[:, 3 + l] = np.asarray(inputs["b2"][l], np.float32)
        bvec[:, 6 + l] = np.asarray(inputs["b3"][l], np.float32) * K / SCALE
        bvec[:, 9 + l] = np.asarray(inputs["di_b"][l], np.float32)
        bvec[:, 12 + l] = np.asarray(inputs["do_b"][l], np.float32)
        lnvec[:, 0 + l] = np.asarray(inputs["n1_s"][l], np.float32)
        lnvec[:, 3 + l] = np.asarray(inputs["n1_b"][l], np.float32)
        lnvec[:, 6 + l] = np.asarray(inputs["n2_s"][l], np.float32)
        lnvec[:, 9 + l] = np.asarray(inputs["n2_b"][l], np.float32)
    shared["bvec"] = bvec
    shared["lnvec"] = lnvec

    in_maps = []
    for c in range(ncores):
        sl = slice(c * nper, (c + 1) * nper)
        efc = ef[sl].astype(bf)                              # (nper, K, C)
        in_maps.append(dict(
            efT=np.ascontiguousarray(efc.transpose(2, 1, 0)),  # (C, K, nper)
            nfT=np.ascontiguousarray(nf[sl].T).astype(bf),
            maskT=mask[sl].reshape(1, nper).astype(bf),
            **shared,
        ))
    return in_maps


_NC_CACHE = {}


def kernel(**inputs):
    in_maps = host_prep(inputs)
    if "nc" not in _NC_CACHE:
        _NC_CACHE["nc"] = build_nc()
    nc = _NC_CACHE["nc"]
    res = run_bass_kernel_spmd(nc, in_maps, core_ids=list(range(NCORES)))
    out = np.concatenate([np.asarray(res.results[c]["out_hT"]).T
                          for c in range(NCORES)], axis=0)
    return np.ascontiguousarray(out.astype(np.float32))


# revision 25
# speedup vs baseline: 1.0352x; 1.0352x over previous
"""Trainium2 Bass kernel for nn_Decoder (3-layer GNN message-passing decoder).

Sharding: node axis split across 8 cores (2500 nodes/core), weights replicated.
Feature axis (C=128) lives on partitions; host pre-transposes to [C, ...] bf16.

v2 design (vs baseline): the LayerNorm rstd is computed on the Vector engine
with custom DVE ops (quadratic seed + 2 Newton steps, ~1.5e-4 rel err) instead
of Ln/Exp on the Scalar engine.  The ACT engine therefore only ever loads the
gelu table set (once), which lets the per-tile node phase (LN1/FFN/LN2) be
software-pipelined into the NEXT tile's edge phase with zero table thrash:
NODE(l,t) is emitted as ~10 small chunks spliced between the matmul/gelu
groups of EDGE(l,t+1).  LN applies and squares run on the otherwise-idle
GPSIMD engine.

Per-core, per-(layer,tile) structure (T=500-node tiles, K=32 edge slots):
  EDGE: S-broadcast add on DVE (PSUM->SBUF stg), gelu1 in 4000-elem ACT
        chunks, mm2 + gelu2 per 2-k PSUM group, mm3 PSUM-accumulated onto
        identity-preloaded h.
  NODE: x = acc + K*b3/30; stats via (1/C)-ones matmuls; var on DVE custom op;
        rstd via RSQRT_SEED + 2x RSQRT_NR (DVE); LN applies + mask on GPSIMD;
        FFN (di/do) matmuls on PE with gelu on ACT; next-layer S on PE.
"""

import sys
from contextlib import ExitStack

for _p in ("/opt/trn_rl_repo", "/root/.axon_site/_ro/trn_rl_repo"):
    if _p not in sys.path:
        sys.path.append(_p)

import numpy as np
import ml_dtypes

import concourse.bass as bass
import concourse.tile as tile
from concourse import bacc, mybir
from concourse.bass_utils import run_bass_kernel_spmd
from concourse.masks import make_identity
from concourse import dve_ops as _dvo
from concourse.dve_spec import (
    C0, C1, C2, Spec, Src0, Src1, lower as _dve_lower, _has_src1,
)
from concourse.dve_uop import DveOpSpec

N, K, C, H, L = 20000, 32, 128, 128, 3
NCORES = 8
NPER = N // NCORES          # 2500 nodes per core
T = 500                     # node tile
NT = NPER // T              # 5 tiles
SCALE, EPS = 30.0, 1e-5

BF = mybir.dt.bfloat16
F32 = mybir.dt.float32
AF = mybir.ActivationFunctionType
OP = mybir.AluOpType

# ---- custom DVE ops (registered once per process) ----

def _register_dve_op(name, spec, subdim=False):
    for o in _dvo.OPS:
        if o.name == name:
            return o
    row = max(_dvo._SUB_OPCODE_FOR_NAME.values()) + 1
    assert row < 0x20, "no free custom-DVE opcode rows"
    _dvo._SUB_OPCODE_FOR_NAME[name] = row
    shas = {}
    for ver in ("v3",):
        uops = _dve_lower(spec, ver=ver)
        shas[ver] = DveOpSpec(
            name=name, opcode=row, uops=uops, rd1_en=_has_src1(spec)
        ).sha(ver)
    op = _dvo.DveOp(name, spec, subdim=subdim, uops_sha=shas)
    _dvo.OPS.append(op)
    _dvo.CUSTOM_DVE_SPECS[name] = spec
    return op


# z0 = (c0*v + c1)*v + c2 : quadratic minimax seed for rsqrt on v in [0.35, 3]
RSQRT_SEED = _register_dve_op(
    "RSQRT_SEED_ANT",
    Spec(
        body=(C0 * Src0 + C1) * Src0 + C2,
        reference=lambda in0, in1, s0, s1, imm2: (
            (s0 * in0 + s1) * in0 + imm2
        ).astype(np.float32),
    ),
)
# z' = z*(1.5 - 0.5*v*z*z) : one rsqrt Newton step (Src0=v, Src1=z)
RSQRT_NR = _register_dve_op(
    "RSQRT_NR_ANT",
    Spec(
        body=Src1 * (C0 - C1 * (Src0 * (Src1 * Src1))),
        reference=lambda in0, in1, s0, s1, imm2: (
            in1 * (s0 - s1 * in0 * in1 * in1)
        ).astype(np.float32),
    ),
)
# u = in1 - in0*in0 + eps : variance from (mean, E[x^2]) with eps folded in
VAR_EPS = _register_dve_op(
    "VAR_EPS_ANT",
    Spec(
        body=(Src1 - Src0 * Src0) + C0,
        reference=lambda in0, in1, s0, s1, imm2: (
            in1 - in0 * in0 + s0
        ).astype(np.float32),
    ),
)

RS_C0, RS_C1, RS_C2 = 0.19236749, -0.99543426, 1.87889486


def _emit(ctx, tc, io, nper, tsz):
    nc = tc.nc
    nt = nper // tsz

    consts = ctx.enter_context(tc.tile_pool(name="consts", bufs=1))
    efpool = ctx.enter_context(tc.tile_pool(name="ef", bufs=2))
    stgpool = ctx.enter_context(tc.tile_pool(name="stg", bufs=2))
    m12pool = ctx.enter_context(tc.tile_pool(name="m12", bufs=2))
    spool = ctx.enter_context(tc.tile_pool(name="sp", bufs=4))
    meanpool = ctx.enter_context(tc.tile_pool(name="mean", bufs=3))
    upool = ctx.enter_context(tc.tile_pool(name="u", bufs=2))
    zpool = ctx.enter_context(tc.tile_pool(name="z", bufs=3))
    sqpool = ctx.enter_context(tc.tile_pool(name="sq", bufs=3))
    gpool = ctx.enter_context(tc.tile_pool(name="g", bufs=4))
    mdpool = ctx.enter_context(tc.tile_pool(name="md", bufs=2))
    outpool = ctx.enter_context(tc.tile_pool(name="out", bufs=2))
    papool = ctx.enter_context(tc.tile_pool(name="pa", bufs=2, space="PSUM"))
    pbpool = ctx.enter_context(tc.tile_pool(name="pb", bufs=2, space="PSUM"))
    psacc = ctx.enter_context(tc.tile_pool(name="psacc", bufs=1, space="PSUM"))
    psmisc = ctx.enter_context(tc.tile_pool(name="psmisc", bufs=1, space="PSUM"))

    # ---- persistent SBUF state ----
    ef00 = efpool.tile([C, K, tsz], BF, tag="ef", name="ef00")
    for q in range(16):
        nc.sync.dma_start(out=ef00[:, q * 2:(q + 1) * 2, :],
                          in_=io["efT"][:, q * 2:(q + 1) * 2, 0:tsz])
    nfh = consts.tile([C, nper], BF, tag="nfh")
    mask_rep = consts.tile([C, nper], BF, tag="maskr")
    h_sb = consts.tile([C, nper], BF, tag="hbuf")
    h1_sb = consts.tile([C, nper], BF, tag="h1")
    x2t = consts.tile([C, nper], BF, tag="x2t")

    # critical-path setup first: nfh + the weights needed for S(0,*) and the
    # first edge tile, so the first gelu lands as early as possible.  Large
    # host->SBUF transfers are split so they spread across DMA queues.
    wts = {}
    for nm in ("w1aT", "w1bT", "w1eT", "w2T", "w3sT", "diwT", "dowT"):
        wts[nm] = consts.tile([C, L, H], BF, tag=nm, name=nm)
    for nm in ("w1eT", "w1aT", "w1bT"):
        nc.sync.dma_start(out=wts[nm][:, 0, :], in_=io[nm][0, :, :])
    for q in range(4):
        qs = slice(q * (nper // 4), (q + 1) * (nper // 4))
        nc.sync.dma_start(out=nfh[:, qs], in_=io["nfT"][:, qs])
    for nm in ("w1aT", "w1bT", "w1eT", "w2T", "w3sT", "diwT", "dowT"):
        for l in range(L):
            if l == 0 and nm in ("w1eT", "w1aT", "w1bT"):
                continue
            nc.sync.dma_start(out=wts[nm][:, l, :], in_=io[nm][l, :, :])
    bvec = consts.tile([C, 15], F32, tag="bvec")
    nc.sync.dma_start(out=bvec[:, :], in_=io["bvec"][:, :])
    lnvec = consts.tile([C, 12], F32, tag="lnvec")
    nc.sync.dma_start(out=lnvec[:, :], in_=io["lnvec"][:, :])

    ident = consts.tile([C, C], BF, tag="ident")
    make_identity(nc, ident[:, :])
    ones_sc = consts.tile([C, C], BF, tag="ones")
    nc.vector.memset(ones_sc[:, :], 1.0 / C)

    _m = io["maskT"]
    for q in range(4):
        q0 = q * (nper // 4)
        qs = slice(q0, q0 + nper // 4)
        _mb = bass.AP(tensor=_m.tensor, offset=_m.offset + q0,
                      ap=[[0, C], [_m.ap[1][0], nper // 4]])
        nc.sync.dma_start(out=mask_rep[:, qs], in_=_mb)

    def bcol(base, l):
        return bvec[:, base + l:base + l + 1]

    def lncol(base, l):
        return lnvec[:, base + l:base + l + 1]

    s_tiles = {}
    ef_tiles = {}
    acc_tiles = {}

    def emit_S(l, t, h_src):
        sl_ = slice(t * tsz, (t + 1) * tsz)
        s_ps = psmisc.tile([C, 512], F32, tag="pm", name="s_ps")
        nc.tensor.matmul(s_ps[:, 0:tsz], wts["w1aT"][:, l, :], h_src[:, sl_],
                         start=True, stop=False)
        nc.tensor.matmul(s_ps[:, 0:tsz], wts["w1bT"][:, l, :], nfh[:, sl_],
                         start=False, stop=True)
        s_sb = spool.tile([C, tsz], BF, tag="ssb", name="s_sb")
        nc.vector.tensor_copy(out=s_sb[:, :], in_=s_ps[:, 0:tsz])
        s_tiles[(l, t)] = s_sb

    def fetch_ef(l, t):
        if (l, t) in ef_tiles or t >= nt:
            return
        sl_ = slice(t * tsz, (t + 1) * tsz)
        ef_sb = efpool.tile([C, K, tsz], BF, tag="ef")
        for q in range(16):
            nc.sync.dma_start(out=ef_sb[:, q * 2:(q + 1) * 2, :],
                              in_=io["efT"][:, q * 2:(q + 1) * 2, sl_])
        ef_tiles[(l, t)] = ef_sb

    # ---- NODE(l, t): list of closures spliced into the next EDGE ----
    def node_chunks(l, t):
        sl = slice(t * tsz, (t + 1) * tsz)
        st = {}

        def ln_stats(src_bf):
            # -> (mean_sb f32, z f32) ; consumes one sq tile via gpsimd
            sq = sqpool.tile([C, tsz], BF, tag="sq", name="sq")
            nc.gpsimd.tensor_mul(sq[:, :], src_bf, src_bf)
            return sq

        def c0():
            acc_ps = acc_tiles.pop((l, t))
            nc.vector.tensor_scalar(x2t[:, sl], acc_ps[:, 0:tsz],
                                    bcol(6, l), None, OP.add)
            st["sq"] = ln_stats(x2t[:, sl])

        def c1():
            stp = psmisc.tile([C, 512], F32, tag="pm", name="st1")
            nc.tensor.matmul(stp[:, 0:tsz], ones_sc[:, :], x2t[:, sl],
                             start=True, stop=True)
            mean = meanpool.tile([C, tsz], F32, tag="mean", name="mean")
            nc.vector.tensor_copy(out=mean[:, :], in_=stp[:, 0:tsz])
            st["mean"] = mean

        def c2():
            stp = psmisc.tile([C, 512], F32, tag="pm", name="st2")
            nc.tensor.matmul(stp[:, 0:tsz], ones_sc[:, :], st["sq"][:, :],
                             start=True, stop=True)
            u = upool.tile([C, tsz], F32, tag="u", name="u")
            nc.vector._custom_dve(VAR_EPS, out=u[:, :], in0=st["mean"][:, :],
                                  in1=stp[:, 0:tsz], s0=EPS)
            z = zpool.tile([C, tsz], F32, tag="z", name="z")
            nc.vector._custom_dve(RSQRT_SEED, out=z[:, :], in0=u[:, :],
                                  s0=RS_C0, s1=RS_C1, imm2=RS_C2)
            nc.vector._custom_dve(RSQRT_NR, out=z[:, :], in0=u[:, :],
                                  in1=z[:, :], s0=1.5, s1=0.5)
            nc.vector._custom_dve(RSQRT_NR, out=z[:, :], in0=u[:, :],
                                  in1=z[:, :], s0=1.5, s1=0.5)
            st["z"] = z

        def c3():
            g1 = gpool.tile([C, tsz], BF, tag="g", name="g1")
            nc.gpsimd.tensor_sub(g1[:, :], x2t[:, sl], st["mean"][:, :])
            nc.gpsimd.tensor_mul(g1[:, :], g1[:, :], st["z"][:, :])
            nc.gpsimd.tensor_scalar(h1_sb[:, sl], g1[:, :],
                                    lncol(0, l), lncol(3, l), OP.mult, OP.add)

        def c4():
            dpa = psmisc.tile([C, 512], F32, tag="pm", name="dpa")
            nc.tensor.matmul(dpa[:, 0:tsz], wts["diwT"][:, l, :], h1_sb[:, sl],
                             start=True, stop=True)
            md = mdpool.tile([C, tsz], BF, tag="md", name="md")
            nc.scalar.activation(out=md[:, :], in_=dpa[:, 0:tsz], func=AF.Gelu,
                                 bias=bcol(9, l))
            st["md"] = md

        def c5():
            dpb = psmisc.tile([C, 512], F32, tag="pm", name="dpb")
            nc.tensor.matmul(dpb[:, 0:tsz], ident[:, :], h1_sb[:, sl],
                             start=True, stop=False)
            nc.tensor.matmul(dpb[:, 0:tsz], wts["dowT"][:, l, :],
                             st["md"][:, :], start=False, stop=True)
            nc.vector.tensor_scalar(x2t[:, sl], dpb[:, 0:tsz],
                                    bcol(12, l), None, OP.add)
            st["sq2"] = ln_stats(x2t[:, sl])

        def c6():
            stp = psmisc.tile([C, 512], F32, tag="pm", name="st1b")
            nc.tensor.matmul(stp[:, 0:tsz], ones_sc[:, :], x2t[:, sl],
                             start=True, stop=True)
            mean = meanpool.tile([C, tsz], F32, tag="mean", name="mean2")
            nc.vector.tensor_copy(out=mean[:, :], in_=stp[:, 0:tsz])
            st["mean2"] = mean

        def c7():
            stp = psmisc.tile([C, 512], F32, tag="pm", name="st2b")
            nc.tensor.matmul(stp[:, 0:tsz], ones_sc[:, :], st["sq2"][:, :],
                             start=True, stop=True)
            u = upool.tile([C, tsz], F32, tag="u", name="u2")
            nc.vector._custom_dve(VAR_EPS, out=u[:, :], in0=st["mean2"][:, :],
                                  in1=stp[:, 0:tsz], s0=EPS)
            z = zpool.tile([C, tsz], F32, tag="z", name="z2")
            nc.vector._custom_dve(RSQRT_SEED, out=z[:, :], in0=u[:, :],
                                  s0=RS_C0, s1=RS_C1, imm2=RS_C2)
            nc.vector._custom_dve(RSQRT_NR, out=z[:, :], in0=u[:, :],
                                  in1=z[:, :], s0=1.5, s1=0.5)
            nc.vector._custom_dve(RSQRT_NR, out=z[:, :], in0=u[:, :],
                                  in1=z[:, :], s0=1.5, s1=0.5)
            st["z2"] = z

        def c8():
            g2 = gpool.tile([C, tsz], BF, tag="g", name="g2")
            nc.gpsimd.tensor_sub(g2[:, :], x2t[:, sl], st["mean2"][:, :])
            nc.gpsimd.tensor_mul(g2[:, :], g2[:, :], st["z2"][:, :])
            if l < L - 1:
                q = gpool.tile([C, tsz], BF, tag="g", name="q")
                nc.gpsimd.tensor_scalar(q[:, :], g2[:, :],
                                        lncol(6, l), lncol(9, l),
                                        OP.mult, OP.add)
                nc.gpsimd.tensor_mul(h_sb[:, sl], q[:, :], mask_rep[:, sl])
            else:
                q = gpool.tile([C, tsz], BF, tag="g", name="q")
                nc.gpsimd.tensor_scalar(q[:, :], g2[:, :],
                                        lncol(6, l), lncol(9, l),
                                        OP.mult, OP.add)
                ot = outpool.tile([C, tsz], F32, tag="ot", name="ot")
                nc.gpsimd.tensor_mul(ot[:, :], q[:, :], mask_rep[:, sl])
                t0 = t * tsz
                for qq in range(4):
                    qs = slice(t0 + qq * (tsz // 4), t0 + (qq + 1) * (tsz // 4))
                    nc.sync.dma_start(out=io["out_hT"][:, qs],
                                      in_=ot[:, qq * (tsz // 4):(qq + 1) * (tsz // 4)])

        def c9():
            if l < L - 1:
                emit_S(l + 1, t, h_sb)

        return [c0, c1, c2, c3, c4, c5, c6, c7, c8, c9]

    pend = []

    def splice():
        if pend:
            pend.pop(0)()

    # ---- EDGE(l, t) ----
    def edge(l, t):
        sl = slice(t * tsz, (t + 1) * tsz)
        # prefetch next tile's ef
        if t + 1 < nt:
            fetch_ef(l, t + 1)
        elif l + 1 < L:
            fetch_ef(l + 1, 0)
        ef_sb = ef_tiles.pop((l, t))
        h_cur = nfh if l == 0 else h_sb
        w1e = wts["w1eT"][:, l, :]
        w2 = wts["w2T"][:, l, :]
        w3s = wts["w3sT"][:, l, :]

        s_sb = s_tiles.pop((l, t))

        m12 = [m12pool.tile([C, 16, tsz], BF, tag="m12", name=f"m12_{h}")
               for h in range(2)]

        def phase_A(h):
            for sg in range(2):
                stg = stgpool.tile([C, 8, tsz], F32, tag="stg", name="stg")
                for j in range(8):
                    k0 = h * 16 + sg * 8 + j
                    pa = papool.tile([C, 512], F32, tag="pa", name="pa")
                    nc.tensor.matmul(pa[:, 0:tsz], w1e, ef_sb[:, k0, :],
                                     start=True, stop=True)
                    nc.vector.tensor_add(stg[:, j, 0:tsz],
                                         pa[:, 0:tsz], s_sb[:, :])
                nc.scalar.activation(out=m12[h][:, sg * 8:(sg + 1) * 8, 0:tsz],
                                     in_=stg[:, :, 0:tsz],
                                     func=AF.Gelu, bias=bcol(0, l))

        def phase_B(h, gr):
            for g in gr:
                pb = pbpool.tile([C, 2, 512], F32, tag="pb", name="pb")
                for i in range(2):
                    nc.tensor.matmul(pb[:, i, 0:tsz], w2,
                                     m12[h][:, g * 2 + i, 0:tsz],
                                     start=True, stop=True)
                nc.scalar.activation(out=m12[h][:, g * 2:(g + 1) * 2, 0:tsz],
                                     in_=pb[:, :, 0:tsz],
                                     func=AF.Gelu, bias=bcol(3, l))

        def phase_C(h, acc):
            for kk in range(16):
                nc.tensor.matmul(acc[:, 0:tsz], w3s, m12[h][:, kk, 0:tsz],
                                 start=False, stop=(h == 1 and kk == 15))

        phase_A(0)
        phase_B(0, range(0, 4))
        splice(); splice()
        phase_B(0, range(4, 8))
        splice(); splice()
        phase_A(1)
        acc_ps = psacc.tile([C, 512], F32, tag="acc", name="acc_ps")
        nc.tensor.matmul(acc_ps[:, 0:tsz], ident[:, :], h_cur[:, sl],
                         start=True, stop=False)
        acc_tiles[(l, t)] = acc_ps
        phase_C(0, acc_ps)
        splice(); splice()
        phase_B(1, range(0, 4))
        splice(); splice()
        phase_B(1, range(4, 8))
        splice(); splice()
        phase_C(1, acc_ps)

    # ---- main emission ----
    ef_tiles[(0, 0)] = ef00
    for t in range(nt):
        emit_S(0, t, nfh)
    for l in range(L):
        for t in range(nt):
            edge(l, t)
            pend.extend(node_chunks(l, t))
    while pend:
        pend.pop(0)()


def build_nc(nper=NPER, tsz=T):
    nc = bacc.Bacc("TRN2", target_bir_lowering=False, debug=False,
                   enable_asserts=False)
    io = {
        "efT": nc.dram_tensor("efT", [C, K, nper], BF, kind="ExternalInput").ap(),
        "nfT": nc.dram_tensor("nfT", [C, nper], BF, kind="ExternalInput").ap(),
        "maskT": nc.dram_tensor("maskT", [1, nper], BF, kind="ExternalInput").ap(),
        "bvec": nc.dram_tensor("bvec", [C, 15], F32, kind="ExternalInput").ap(),
        "lnvec": nc.dram_tensor("lnvec", [C, 12], F32, kind="ExternalInput").ap(),
        "out_hT": nc.dram_tensor("out_hT", [C, nper], F32, kind="ExternalOutput").ap(),
    }
    for nm in ("w1aT", "w1bT", "w1eT", "w2T", "w3sT", "diwT", "dowT"):
        io[nm] = nc.dram_tensor(nm, [L, C, H], BF, kind="ExternalInput").ap()
    with tile.TileContext(nc) as tc:
        with ExitStack() as ctx:
            _emit(ctx, tc, io, nper, tsz)
    nc.compile()
    return nc


def host_prep(inputs, nper=NPER, ncores=NCORES):
    """Shard + lay out inputs for the device. Returns list of per-core in_maps."""
    bf = ml_dtypes.bfloat16
    nf = np.asarray(inputs["node_features"], np.float32)
    ef = np.asarray(inputs["edge_features"], np.float32)
    mask = np.asarray(inputs["mask"], np.float32)
    w1 = np.asarray(inputs["w1"], np.float32)
    w2 = np.asarray(inputs["w2"], np.float32)
    w3 = np.asarray(inputs["w3"], np.float32)
    di_w = np.asarray(inputs["di_w"], np.float32)
    do_w = np.asarray(inputs["do_w"], np.float32)

    def tr(w):  # (L, A, B) -> (L, B, A) contiguous bf16
        return np.ascontiguousarray(w.transpose(0, 2, 1)).astype(bf)

    shared = {
        "w1aT": tr(w1[:, :, 0:C]),
        "w1bT": tr(w1[:, :, C:2 * C]),
        "w1eT": tr(w1[:, :, 3 * C:4 * C]),
        "w2T": tr(w2),
        "w3sT": tr(w3 / SCALE),
        "diwT": tr(di_w),
        "dowT": tr(do_w),
    }
    bvec = np.zeros((C, 15), np.float32)
    lnvec = np.zeros((C, 12), np.float32)
    for l in range(L):
        bvec[:, 0 + l] = np.asarray(inputs["b1"][l], np.float32)
        bvec[:, 3 + l] = np.asarray(inputs["b2"][l], np.float32)
        bvec[:, 6 + l] = np.asarray(inputs["b3"][l], np.float32) * K / SCALE
        bvec[:, 9 + l] = np.asarray(inputs["di_b"][l], np.float32)
        bvec[:, 12 + l] = np.asarray(inputs["do_b"][l], np.float32)
        lnvec[:, 0 + l] = np.asarray(inputs["n1_s"][l], np.float32)
        lnvec[:, 3 + l] = np.asarray(inputs["n1_b"][l], np.float32)
        lnvec[:, 6 + l] = np.asarray(inputs["n2_s"][l], np.float32)
        lnvec[:, 9 + l] = np.asarray(inputs["n2_b"][l], np.float32)
    shared["bvec"] = bvec
    shared["lnvec"] = lnvec

    in_maps = []
    for c in range(ncores):
        sl = slice(c * nper, (c + 1) * nper)
        efc = ef[sl].astype(bf)                              # (nper, K, C)
        in_maps.append(dict(
            efT=np.ascontiguousarray(efc.transpose(2, 1, 0)),  # (C, K, nper)
            nfT=np.ascontiguousarray(nf[sl].T).astype(bf),
            maskT=mask[sl].reshape(1, nper).astype(bf),
            **shared,
        ))
    return in_maps


_NC_CACHE = {}


def kernel(**inputs):
    in_maps = host_prep(inputs)
    if "nc" not in _NC_CACHE:
        _NC_CACHE["nc"] = build_nc()
    nc = _NC_CACHE["nc"]
    res = run_bass_kernel_spmd(nc, in_maps, core_ids=list(range(NCORES)))
    out = np.concatenate([np.asarray(res.results[c]["out_hT"]).T
                          for c in range(NCORES)], axis=0)
    return np.ascontiguousarray(out.astype(np.float32))


# revision 26
# speedup vs baseline: 1.0493x; 1.0137x over previous
"""Trainium2 Bass kernel for nn_Decoder (3-layer GNN message-passing decoder).

Sharding: node axis split across 8 cores (2500 nodes/core), weights replicated.
Feature axis (C=128) lives on partitions; host pre-transposes to [C, ...] bf16.

v2 design (vs baseline): the LayerNorm rstd is computed on the Vector engine
with custom DVE ops (quadratic seed + 2 Newton steps, ~1.5e-4 rel err) instead
of Ln/Exp on the Scalar engine.  The ACT engine therefore only ever loads the
gelu table set (once), which lets the per-tile node phase (LN1/FFN/LN2) be
software-pipelined into the NEXT tile's edge phase with zero table thrash:
NODE(l,t) is emitted as ~10 small chunks spliced between the matmul/gelu
groups of EDGE(l,t+1).  LN applies and squares run on the otherwise-idle
GPSIMD engine.

Per-core, per-(layer,tile) structure (T=500-node tiles, K=32 edge slots):
  EDGE: S-broadcast add on DVE (PSUM->SBUF stg), gelu1 in 4000-elem ACT
        chunks, mm2 + gelu2 per 2-k PSUM group, mm3 PSUM-accumulated onto
        identity-preloaded h.
  NODE: x = acc + K*b3/30; stats via (1/C)-ones matmuls; var on DVE custom op;
        rstd via RSQRT_SEED + 2x RSQRT_NR (DVE); LN applies + mask on GPSIMD;
        FFN (di/do) matmuls on PE with gelu on ACT; next-layer S on PE.
"""

import sys
from contextlib import ExitStack

for _p in ("/opt/trn_rl_repo", "/root/.axon_site/_ro/trn_rl_repo"):
    if _p not in sys.path:
        sys.path.append(_p)

import numpy as np
import ml_dtypes

import concourse.bass as bass
import concourse.tile as tile
from concourse import bacc, mybir
from concourse.bass_utils import run_bass_kernel_spmd
from concourse.masks import make_identity
from concourse import dve_ops as _dvo
from concourse.dve_spec import (
    C0, C1, C2, Spec, Src0, Src1, lower as _dve_lower, _has_src1,
)
from concourse.dve_uop import DveOpSpec

N, K, C, H, L = 20000, 32, 128, 128, 3
NCORES = 8
NPER = N // NCORES          # 2500 nodes per core
T = 500                     # node tile
NT = NPER // T              # 5 tiles
SCALE, EPS = 30.0, 1e-5

BF = mybir.dt.bfloat16
F32 = mybir.dt.float32
AF = mybir.ActivationFunctionType
OP = mybir.AluOpType

# ---- custom DVE ops (registered once per process) ----

def _register_dve_op(name, spec, subdim=False):
    for o in _dvo.OPS:
        if o.name == name:
            return o
    row = max(_dvo._SUB_OPCODE_FOR_NAME.values()) + 1
    assert row < 0x20, "no free custom-DVE opcode rows"
    _dvo._SUB_OPCODE_FOR_NAME[name] = row
    shas = {}
    for ver in ("v3",):
        uops = _dve_lower(spec, ver=ver)
        shas[ver] = DveOpSpec(
            name=name, opcode=row, uops=uops, rd1_en=_has_src1(spec)
        ).sha(ver)
    op = _dvo.DveOp(name, spec, subdim=subdim, uops_sha=shas)
    _dvo.OPS.append(op)
    _dvo.CUSTOM_DVE_SPECS[name] = spec
    return op


# z0 = (c0*v + c1)*v + c2 : quadratic minimax seed for rsqrt on v in [0.35, 3]
RSQRT_SEED = _register_dve_op(
    "RSQRT_SEED_ANT",
    Spec(
        body=(C0 * Src0 + C1) * Src0 + C2,
        reference=lambda in0, in1, s0, s1, imm2: (
            (s0 * in0 + s1) * in0 + imm2
        ).astype(np.float32),
    ),
)
# z' = z*(1.5 - 0.5*v*z*z) : one rsqrt Newton step (Src0=v, Src1=z)
RSQRT_NR = _register_dve_op(
    "RSQRT_NR_ANT",
    Spec(
        body=Src1 * (C0 - C1 * (Src0 * (Src1 * Src1))),
        reference=lambda in0, in1, s0, s1, imm2: (
            in1 * (s0 - s1 * in0 * in1 * in1)
        ).astype(np.float32),
    ),
)
# u = in1 - in0*in0 + eps : variance from (mean, E[x^2]) with eps folded in
VAR_EPS = _register_dve_op(
    "VAR_EPS_ANT",
    Spec(
        body=(Src1 - Src0 * Src0) + C0,
        reference=lambda in0, in1, s0, s1, imm2: (
            in1 - in0 * in0 + s0
        ).astype(np.float32),
    ),
)

RS_C0, RS_C1, RS_C2 = 0.19236749, -0.99543426, 1.87889486


def _emit(ctx, tc, io, nper, tsz):
    nc = tc.nc
    nt = nper // tsz

    consts = ctx.enter_context(tc.tile_pool(name="consts", bufs=1))
    efpool = ctx.enter_context(tc.tile_pool(name="ef", bufs=2))
    stgpool = ctx.enter_context(tc.tile_pool(name="stg", bufs=2))
    m12pool = ctx.enter_context(tc.tile_pool(name="m12", bufs=2))
    spool = ctx.enter_context(tc.tile_pool(name="sp", bufs=4))
    meanpool = ctx.enter_context(tc.tile_pool(name="mean", bufs=3))
    upool = ctx.enter_context(tc.tile_pool(name="u", bufs=2))
    zpool = ctx.enter_context(tc.tile_pool(name="z", bufs=3))
    sqpool = ctx.enter_context(tc.tile_pool(name="sq", bufs=3))
    gpool = ctx.enter_context(tc.tile_pool(name="g", bufs=4))
    mdpool = ctx.enter_context(tc.tile_pool(name="md", bufs=2))
    outpool = ctx.enter_context(tc.tile_pool(name="out", bufs=2))
    papool = ctx.enter_context(tc.tile_pool(name="pa", bufs=2, space="PSUM"))
    pbpool = ctx.enter_context(tc.tile_pool(name="pb", bufs=2, space="PSUM"))
    psacc = ctx.enter_context(tc.tile_pool(name="psacc", bufs=1, space="PSUM"))
    psmisc = ctx.enter_context(tc.tile_pool(name="psmisc", bufs=1, space="PSUM"))

    # ---- persistent SBUF state ----
    warm = consts.tile([C, 1], F32, tag="warm")
    nc.vector.memset(warm[:, :], 0.0)
    nc.scalar.activation(out=warm[:, :], in_=warm[:, :], func=AF.Gelu)
    ef00 = efpool.tile([C, K, tsz], BF, tag="ef", name="ef00")
    for q in range(16):
        nc.sync.dma_start(out=ef00[:, q * 2:(q + 1) * 2, :],
                          in_=io["efT"][:, q * 2:(q + 1) * 2, 0:tsz])
    nfh = consts.tile([C, nper], BF, tag="nfh")
    mask_rep = consts.tile([C, nper], BF, tag="maskr")
    h_sb = consts.tile([C, nper], BF, tag="hbuf")
    h1_sb = consts.tile([C, nper], BF, tag="h1")
    x2t = consts.tile([C, nper], BF, tag="x2t")

    # critical-path setup first: nfh + the weights needed for S(0,*) and the
    # first edge tile, so the first gelu lands as early as possible.  Large
    # host->SBUF transfers are split so they spread across DMA queues.
    wts = {}
    for nm in ("w1aT", "w1bT", "w1eT", "w2T", "w3sT", "diwT", "dowT"):
        wts[nm] = consts.tile([C, L, H], BF, tag=nm, name=nm)
    for nm in ("w1eT", "w1aT", "w1bT"):
        nc.sync.dma_start(out=wts[nm][:, 0, :], in_=io[nm][0, :, :])
    for q in range(4):
        qs = slice(q * (nper // 4), (q + 1) * (nper // 4))
        nc.sync.dma_start(out=nfh[:, qs], in_=io["nfT"][:, qs])
    for nm in ("w1aT", "w1bT", "w1eT", "w2T", "w3sT", "diwT", "dowT"):
        for l in range(L):
            if l == 0 and nm in ("w1eT", "w1aT", "w1bT"):
                continue
            nc.sync.dma_start(out=wts[nm][:, l, :], in_=io[nm][l, :, :])
    bvec = consts.tile([C, 15], F32, tag="bvec")
    nc.sync.dma_start(out=bvec[:, :], in_=io["bvec"][:, :])
    lnvec = consts.tile([C, 12], F32, tag="lnvec")
    nc.sync.dma_start(out=lnvec[:, :], in_=io["lnvec"][:, :])

    ident = consts.tile([C, C], BF, tag="ident")
    make_identity(nc, ident[:, :])
    ones_sc = consts.tile([C, C], BF, tag="ones")
    nc.vector.memset(ones_sc[:, :], 1.0 / C)

    _m = io["maskT"]
    for q in range(4):
        q0 = q * (nper // 4)
        qs = slice(q0, q0 + nper // 4)
        _mb = bass.AP(tensor=_m.tensor, offset=_m.offset + q0,
                      ap=[[0, C], [_m.ap[1][0], nper // 4]])
        nc.sync.dma_start(out=mask_rep[:, qs], in_=_mb)

    def bcol(base, l):
        return bvec[:, base + l:base + l + 1]

    def lncol(base, l):
        return lnvec[:, base + l:base + l + 1]

    s_tiles = {}
    ef_tiles = {}
    acc_tiles = {}

    def emit_S(l, t, h_src):
        sl_ = slice(t * tsz, (t + 1) * tsz)
        s_ps = psmisc.tile([C, 512], F32, tag="pm", name="s_ps")
        nc.tensor.matmul(s_ps[:, 0:tsz], wts["w1aT"][:, l, :], h_src[:, sl_],
                         start=True, stop=False)
        nc.tensor.matmul(s_ps[:, 0:tsz], wts["w1bT"][:, l, :], nfh[:, sl_],
                         start=False, stop=True)
        s_sb = spool.tile([C, tsz], BF, tag="ssb", name="s_sb")
        nc.vector.tensor_copy(out=s_sb[:, :], in_=s_ps[:, 0:tsz])
        s_tiles[(l, t)] = s_sb

    def fetch_ef(l, t):
        if (l, t) in ef_tiles or t >= nt:
            return
        sl_ = slice(t * tsz, (t + 1) * tsz)
        ef_sb = efpool.tile([C, K, tsz], BF, tag="ef")
        for q in range(16):
            nc.sync.dma_start(out=ef_sb[:, q * 2:(q + 1) * 2, :],
                              in_=io["efT"][:, q * 2:(q + 1) * 2, sl_])
        ef_tiles[(l, t)] = ef_sb

    # ---- NODE(l, t): list of closures spliced into the next EDGE ----
    def node_chunks(l, t, part=None):
        if part is None:
            sl = slice(t * tsz, (t + 1) * tsz)
        else:
            h2 = tsz // 2
            sl = slice(t * tsz + part * h2, t * tsz + (part + 1) * h2)
        npts = sl.stop - sl.start
        st = {}

        def ln_stats(src_bf):
            # -> (mean_sb f32, z f32) ; consumes one sq tile via gpsimd
            sq = sqpool.tile([C, npts], BF, tag="sq", name="sq")
            nc.gpsimd.tensor_mul(sq[:, :], src_bf, src_bf)
            return sq

        def c0():
            acc_ps = acc_tiles[(l, t)]
            a0 = 0 if part is None else part * (tsz // 2)
            nc.vector.tensor_scalar(x2t[:, sl], acc_ps[:, a0:a0 + npts],
                                    bcol(6, l), None, OP.add)
            st["sq"] = ln_stats(x2t[:, sl])

        def c1():
            stp = psmisc.tile([C, 512], F32, tag="pm", name="st1")
            nc.tensor.matmul(stp[:, 0:npts], ones_sc[:, :], x2t[:, sl],
                             start=True, stop=True)
            mean = meanpool.tile([C, npts], F32, tag="mean", name="mean")
            nc.vector.tensor_copy(out=mean[:, :], in_=stp[:, 0:npts])
            st["mean"] = mean

        def c2():
            stp = psmisc.tile([C, 512], F32, tag="pm", name="st2")
            nc.tensor.matmul(stp[:, 0:npts], ones_sc[:, :], st["sq"][:, :],
                             start=True, stop=True)
            u = upool.tile([C, npts], F32, tag="u", name="u")
            nc.vector._custom_dve(VAR_EPS, out=u[:, :], in0=st["mean"][:, :],
                                  in1=stp[:, 0:npts], s0=EPS)
            z = zpool.tile([C, npts], F32, tag="z", name="z")
            nc.vector._custom_dve(RSQRT_SEED, out=z[:, :], in0=u[:, :],
                                  s0=RS_C0, s1=RS_C1, imm2=RS_C2)
            nc.vector._custom_dve(RSQRT_NR, out=z[:, :], in0=u[:, :],
                                  in1=z[:, :], s0=1.5, s1=0.5)
            nc.vector._custom_dve(RSQRT_NR, out=z[:, :], in0=u[:, :],
                                  in1=z[:, :], s0=1.5, s1=0.5)
            st["z"] = z

        def c3():
            g1 = gpool.tile([C, npts], BF, tag="g", name="g1")
            nc.gpsimd.tensor_sub(g1[:, :], x2t[:, sl], st["mean"][:, :])
            nc.gpsimd.tensor_mul(g1[:, :], g1[:, :], st["z"][:, :])
            nc.gpsimd.tensor_scalar(h1_sb[:, sl], g1[:, :],
                                    lncol(0, l), lncol(3, l), OP.mult, OP.add)

        def c4():
            dpa = psmisc.tile([C, 512], F32, tag="pm", name="dpa")
            nc.tensor.matmul(dpa[:, 0:npts], wts["diwT"][:, l, :], h1_sb[:, sl],
                             start=True, stop=True)
            md = mdpool.tile([C, npts], BF, tag="md", name="md")
            nc.scalar.activation(out=md[:, :], in_=dpa[:, 0:npts], func=AF.Gelu,
                                 bias=bcol(9, l))
            st["md"] = md

        def c5():
            dpb = psmisc.tile([C, 512], F32, tag="pm", name="dpb")
            nc.tensor.matmul(dpb[:, 0:npts], ident[:, :], h1_sb[:, sl],
                             start=True, stop=False)
            nc.tensor.matmul(dpb[:, 0:npts], wts["dowT"][:, l, :],
                             st["md"][:, :], start=False, stop=True)
            nc.vector.tensor_scalar(x2t[:, sl], dpb[:, 0:npts],
                                    bcol(12, l), None, OP.add)
            st["sq2"] = ln_stats(x2t[:, sl])

        def c6():
            stp = psmisc.tile([C, 512], F32, tag="pm", name="st1b")
            nc.tensor.matmul(stp[:, 0:npts], ones_sc[:, :], x2t[:, sl],
                             start=True, stop=True)
            mean = meanpool.tile([C, npts], F32, tag="mean", name="mean2")
            nc.vector.tensor_copy(out=mean[:, :], in_=stp[:, 0:npts])
            st["mean2"] = mean

        def c7():
            stp = psmisc.tile([C, 512], F32, tag="pm", name="st2b")
            nc.tensor.matmul(stp[:, 0:npts], ones_sc[:, :], st["sq2"][:, :],
                             start=True, stop=True)
            u = upool.tile([C, npts], F32, tag="u", name="u2")
            nc.vector._custom_dve(VAR_EPS, out=u[:, :], in0=st["mean2"][:, :],
                                  in1=stp[:, 0:npts], s0=EPS)
            z = zpool.tile([C, npts], F32, tag="z", name="z2")
            nc.vector._custom_dve(RSQRT_SEED, out=z[:, :], in0=u[:, :],
                                  s0=RS_C0, s1=RS_C1, imm2=RS_C2)
            nc.vector._custom_dve(RSQRT_NR, out=z[:, :], in0=u[:, :],
                                  in1=z[:, :], s0=1.5, s1=0.5)
            nc.vector._custom_dve(RSQRT_NR, out=z[:, :], in0=u[:, :],
                                  in1=z[:, :], s0=1.5, s1=0.5)
            st["z2"] = z

        def c8():
            g2 = gpool.tile([C, npts], BF, tag="g", name="g2")
            nc.gpsimd.tensor_sub(g2[:, :], x2t[:, sl], st["mean2"][:, :])
            nc.gpsimd.tensor_mul(g2[:, :], g2[:, :], st["z2"][:, :])
            if l < L - 1:
                q = gpool.tile([C, npts], BF, tag="g", name="q")
                nc.gpsimd.tensor_scalar(q[:, :], g2[:, :],
                                        lncol(6, l), lncol(9, l),
                                        OP.mult, OP.add)
                nc.gpsimd.tensor_mul(h_sb[:, sl], q[:, :], mask_rep[:, sl])
            else:
                q = gpool.tile([C, npts], BF, tag="g", name="q")
                nc.gpsimd.tensor_scalar(q[:, :], g2[:, :],
                                        lncol(6, l), lncol(9, l),
                                        OP.mult, OP.add)
                ot = outpool.tile([C, npts], F32, tag="ot", name="ot")
                nc.gpsimd.tensor_mul(ot[:, :], q[:, :], mask_rep[:, sl])
                t0 = sl.start
                nq = 4 if part is None else 2
                for qq in range(nq):
                    cw = npts // nq
                    qs = slice(t0 + qq * cw, t0 + (qq + 1) * cw)
                    nc.sync.dma_start(out=io["out_hT"][:, qs],
                                      in_=ot[:, qq * cw:(qq + 1) * cw])

        def c9():
            if l < L - 1:
                emit_S(l + 1, t, h_sb)

        return [c0, c1, c2, c3, c4, c5, c6, c7, c8, c9]

    pend = []

    def splice():
        if pend:
            pend.pop(0)()

    # ---- EDGE(l, t) ----
    def edge(l, t):
        sl = slice(t * tsz, (t + 1) * tsz)
        # prefetch next tile's ef
        if t + 1 < nt:
            fetch_ef(l, t + 1)
        elif l + 1 < L:
            fetch_ef(l + 1, 0)
        ef_sb = ef_tiles.pop((l, t))
        h_cur = nfh if l == 0 else h_sb
        w1e = wts["w1eT"][:, l, :]
        w2 = wts["w2T"][:, l, :]
        w3s = wts["w3sT"][:, l, :]

        s_sb = s_tiles.pop((l, t))

        m12 = [m12pool.tile([C, 16, tsz], BF, tag="m12", name=f"m12_{h}")
               for h in range(2)]

        def phase_A(h):
            for sg in range(2):
                stg = stgpool.tile([C, 8, tsz], F32, tag="stg", name="stg")
                for j in range(8):
                    k0 = h * 16 + sg * 8 + j
                    pa = papool.tile([C, 512], F32, tag="pa", name="pa")
                    nc.tensor.matmul(pa[:, 0:tsz], w1e, ef_sb[:, k0, :],
                                     start=True, stop=True)
                    nc.vector.tensor_add(stg[:, j, 0:tsz],
                                         pa[:, 0:tsz], s_sb[:, :])
                nc.scalar.activation(out=m12[h][:, sg * 8:(sg + 1) * 8, 0:tsz],
                                     in_=stg[:, :, 0:tsz],
                                     func=AF.Gelu, bias=bcol(0, l))

        def phase_B(h, gr):
            for g in gr:
                pb = pbpool.tile([C, 2, 512], F32, tag="pb", name="pb")
                for i in range(2):
                    nc.tensor.matmul(pb[:, i, 0:tsz], w2,
                                     m12[h][:, g * 2 + i, 0:tsz],
                                     start=True, stop=True)
                nc.scalar.activation(out=m12[h][:, g * 2:(g + 1) * 2, 0:tsz],
                                     in_=pb[:, :, 0:tsz],
                                     func=AF.Gelu, bias=bcol(3, l))

        def phase_C(h, acc):
            for kk in range(16):
                nc.tensor.matmul(acc[:, 0:tsz], w3s, m12[h][:, kk, 0:tsz],
                                 start=False, stop=(h == 1 and kk == 15))

        phase_A(0)
        phase_B(0, range(0, 4))
        splice(); splice()
        phase_B(0, range(4, 8))
        splice(); splice()
        phase_A(1)
        acc_ps = psacc.tile([C, 512], F32, tag="acc", name="acc_ps")
        nc.tensor.matmul(acc_ps[:, 0:tsz], ident[:, :], h_cur[:, sl],
                         start=True, stop=False)
        acc_tiles[(l, t)] = acc_ps
        phase_C(0, acc_ps)
        splice(); splice()
        phase_B(1, range(0, 4))
        splice(); splice()
        phase_B(1, range(4, 8))
        splice(); splice()
        phase_C(1, acc_ps)

    # ---- main emission ----
    ef_tiles[(0, 0)] = ef00
    for t in range(nt):
        emit_S(0, t, nfh)
    for l in range(L):
        for t in range(nt):
            edge(l, t)
            if l == L - 1 and t == nt - 1:
                ca = node_chunks(l, t, part=0)
                cb = node_chunks(l, t, part=1)
                inter = [c for pair in zip(ca, cb) for c in pair]
                pend.extend(inter)
            else:
                pend.extend(node_chunks(l, t))
    while pend:
        pend.pop(0)()


def build_nc(nper=NPER, tsz=T):
    nc = bacc.Bacc("TRN2", target_bir_lowering=False, debug=False,
                   enable_asserts=False)
    io = {
        "efT": nc.dram_tensor("efT", [C, K, nper], BF, kind="ExternalInput").ap(),
        "nfT": nc.dram_tensor("nfT", [C, nper], BF, kind="ExternalInput").ap(),
        "maskT": nc.dram_tensor("maskT", [1, nper], BF, kind="ExternalInput").ap(),
        "bvec": nc.dram_tensor("bvec", [C, 15], F32, kind="ExternalInput").ap(),
        "lnvec": nc.dram_tensor("lnvec", [C, 12], F32, kind="ExternalInput").ap(),
        "out_hT": nc.dram_tensor("out_hT", [C, nper], F32, kind="ExternalOutput").ap(),
    }
    for nm in ("w1aT", "w1bT", "w1eT", "w2T", "w3sT", "diwT", "dowT"):
        io[nm] = nc.dram_tensor(nm, [L, C, H], BF, kind="ExternalInput").ap()
    with tile.TileContext(nc) as tc:
        with ExitStack() as ctx:
            _emit(ctx, tc, io, nper, tsz)
    nc.compile()
    return nc


def host_prep(inputs, nper=NPER, ncores=NCORES):
    """Shard + lay out inputs for the device. Returns list of per-core in_maps."""
    bf = ml_dtypes.bfloat16
    nf = np.asarray(inputs["node_features"], np.float32)
    ef = np.asarray(inputs["edge_features"], np.float32)
    mask = np.asarray(inputs["mask"], np.float32)
    w1 = np.asarray(inputs["w1"], np.float32)
    w2 = np.asarray(inputs["w2"], np.float32)
    w3 = np.asarray(inputs["w3"], np.float32)
    di_w = np.asarray(inputs["di_w"], np.float32)
    do_w = np.asarray(inputs["do_w"], np.float32)

    def tr(w):  # (L, A, B) -> (L, B, A) contiguous bf16
        return np.ascontiguousarray(w.transpose(0, 2, 1)).astype(bf)

    shared = {
        "w1aT": tr(w1[:, :, 0:C]),
        "w1bT": tr(w1[:, :, C:2 * C]),
        "w1eT": tr(w1[:, :, 3 * C:4 * C]),
        "w2T": tr(w2),
        "w3sT": tr(w3 / SCALE),
        "diwT": tr(di_w),
        "dowT": tr(do_w),
    }
    bvec = np.zeros((C, 15), np.float32)
    lnvec = np.zeros((C, 12), np.float32)
    for l in range(L):
        bvec[:, 0 + l] = np.asarray(inputs["b1"][l], np.float32)
        bvec[:, 3 + l] = np.asarray(inputs["b2"][l], np.float32)
        bvec[:, 6 + l] = np.asarray(inputs["b3"][l], np.float32) * K / SCALE
        bvec[:, 9 + l] = np.asarray(inputs["di_b"][l], np.float32)
        bvec[:, 12 + l] = np.asarray(inputs["do_b"][l], np.float32)
        lnvec[:, 0 + l] = np.asarray(inputs["n1_s"][l], np.float32)
        lnvec[:, 3 + l] = np.asarray(inputs["n1_b"][l], np.float32)
        lnvec[:, 6 + l] = np.asarray(inputs["n2_s"][l], np.float32)
        lnvec[:, 9 + l] = np.asarray(inputs["n2_b"][l], np.float32)
    shared["bvec"] = bvec
    shared["lnvec"] = lnvec

    in_maps = []
    for c in range(ncores):
        sl = slice(c * nper, (c + 1) * nper)
        efc = ef[sl].astype(bf)                              # (nper, K, C)
        in_maps.append(dict(
            efT=np.ascontiguousarray(efc.transpose(2, 1, 0)),  # (C, K, nper)
            nfT=np.ascontiguousarray(nf[sl].T).astype(bf),
            maskT=mask[sl].reshape(1, nper).astype(bf),
            **shared,
        ))
    return in_maps


_NC_CACHE = {}


def kernel(**inputs):
    in_maps = host_prep(inputs)
    if "nc" not in _NC_CACHE:
        _NC_CACHE["nc"] = build_nc()
    nc = _NC_CACHE["nc"]
    res = run_bass_kernel_spmd(nc, in_maps, core_ids=list(range(NCORES)))
    out = np.concatenate([np.asarray(res.results[c]["out_hT"]).T
                          for c in range(NCORES)], axis=0)
    return np.ascontiguousarray(out.astype(np.float32))
